# revision 1
# baseline (speedup 1.0000x reference)
"""Trainium2 Bass kernel for nn_Brick_Wall (brick-wall gate-layer gradient).

Math: for each gate g the 4x4 antisymmetric E(chi) splits over so(4) =
su(2)+su(2) as E = L(a) + R(b) (left/right quaternion multiplications), so
expm(E) = L(exp_H a) R(exp_H b) in closed form (sin/cos), and the Frechet
derivative d expm(E)[D_m] contracts against the per-gate matrix
Z = (W C^T - C^T W) U down to two per-gate 4-vectors kappa/lambda:
    partials[m] = dp_m . kappa + dq_m . lambda
with dp_m/dq_m given by the derivative of the quaternion exp in closed form.

Sharding: gates (2048) split contiguously across 8 cores (256 = 2 blocks of
128 partitions each). Host does layout marshaling only (diag-block extraction,
signed column permutations, reshapes); all arithmetic runs on-device.
"""
import sys

for _p in ("/opt/trn_rl_repo",):
    if _p not in sys.path:
        sys.path.insert(0, _p)

import numpy as np

import concourse.bacc as bacc
import concourse.bass as bass
import concourse.tile as tile
from concourse import mybir
from concourse.bass_utils import run_bass_kernel_spmd

F32 = np.float32
P = 128          # partitions (gates per block)
B = 2            # gate blocks per core
NCORES = 8
GPC = P * B      # gates per core
PI = float(np.pi)
DT = mybir.dt.float32

# ---------------- constant tables (quaternion algebra) ----------------
_Q = np.zeros((4, 4, 4))
for (a, b), (c, s) in {
    (0, 0): (0, 1), (0, 1): (1, 1), (0, 2): (2, 1), (0, 3): (3, 1),
    (1, 0): (1, 1), (1, 1): (0, -1), (1, 2): (3, 1), (1, 3): (2, -1),
    (2, 0): (2, 1), (2, 1): (3, -1), (2, 2): (0, -1), (2, 3): (1, 1),
    (3, 0): (3, 1), (3, 1): (2, 1), (3, 2): (1, -1), (3, 3): (0, -1),
}.items():
    _Q[a, b, c] = s

G_SGN = np.zeros((4, 4))   # R(qbar)[k,j] = G_SGN[k,j] * q_{k xor j}
H_SGN = np.zeros((4, 4))   # L(pbar)[i,k] = H_SGN[k,i] * p_{i xor k}
SL = np.zeros((4, 4))      # kappa_a = sum_j SL[a^j, j] * G[a^j, j]
SR = np.zeros((4, 4))      # lambda_b = sum_j SR[b^j, j] * H[b^j, j]
for k in range(4):
    for j in range(4):
        a = k ^ j
        G_SGN[k, j] = _Q[j, a, k] * (1 if a == 0 else -1)
        H_SGN[k, j] = _Q[a, k, j] * (1 if a == 0 else -1)
for a in range(4):
    for j in range(4):
        SL[a ^ j, j] = _Q[a, j, a ^ j]
for b in range(4):
    for j in range(4):
        SR[b ^ j, j] = _Q[j, b, b ^ j]

# internal direction order m' -> chi index; c(m')-1 = (0,0,1,1,2,2)
MPRIME = [4, 5, 1, 2, 0, 3]
SA = [1.0, 1.0, -1.0, -1.0, 1.0, -1.0]
SB = [1.0, -1.0, 1.0, -1.0, -1.0, -1.0]

# XOR gather: row k of the idx table (k^0, k^1, k^2, k^3) as offset + 2D AP
XOR_AP = {0: (0, 2, 1), 1: (1, 2, -1), 2: (2, -2, 1), 3: (3, -2, -1)}
# kappa/lambda reduce position sets {4*(a^j)+j} as offset + 2D strides
KPOS_AP = {0: (0, 10, 5), 1: (1, 10, 3), 2: (2, 6, 5), 3: (3, 6, 3)}

# const row layout (width NC1): SL[0:16] SR[16:32] SA[32:38] SB[38:44]
# G_SGN[44:60] H_SGN[60:76] -pi[76] pi/2[77] sign8[78:86]
NC1 = 86
AB_OFF, PP_OFF, CST_OFF = 0, 12, 16     # layout: ab(12) pp(4) cst(NC1) cb(32) ub(32)
CB_OFF = CST_OFF + NC1                   # 102
UB_OFF = CB_OFF + 32                     # 134
IN1_W = UB_OFF + 32                      # 166


def _const_row() -> np.ndarray:
    c = np.zeros((1, NC1), F32)
    c[0, 0:16] = SL.reshape(16)
    c[0, 16:32] = SR.reshape(16)
    c[0, 32:38] = SA
    c[0, 38:44] = SB
    c[0, 44:60] = G_SGN.reshape(16)
    c[0, 60:76] = H_SGN.reshape(16)
    c[0, 76] = -PI
    c[0, 77] = PI / 2
    c[0, 78:82] = 1.0
    c[0, 82:86] = -1.0
    return c


def _ap(base: bass.AP, off: int, *dims) -> bass.AP:
    """Rebuild an AP over `base`'s tensor: partition dim kept, free dims given
    as (stride, size) pairs, offset in elements added to base offset."""
    return bass.AP(tensor=base.tensor, offset=base.offset + off,
                   ap=[base.ap[0]] + [[s, n] for (s, n) in dims])


def tile_body(ctx, tc, outs, ins):
    """ins: in1(128, IN1_W) = ab|pp|cst, in2(128, IN2_W) = cb|ub
    outs: res(128,12)  [B,6] per partition, internal m' order, sign-applied."""
    nc = tc.nc
    A = mybir.AluOpType
    AF = mybir.ActivationFunctionType
    (in1_d,) = ins
    res_d = outs[0]

    pool = ctx.enter_context(tc.tile_pool(name="main", bufs=1))

    def T(tag, *shape):
        return pool.tile([P, *shape], DT, tag=tag, name=tag)

    # ---- DMA in ----
    in1 = T("in1", IN1_W)
    in2 = in1
    nc.sync.dma_start(in1[:], in1_d)
    CS = CST_OFF
    cst = in1
    hpi = cst[:, CS + 77:CS + 78]

    # ---- S1: w = [a; b] = [al+be; al-be]   w[B,2,3] ----
    w = T("w", B, 2, 3)
    nc.vector.tensor_add(w[:, :, 0, :], _ap(in1[:], AB_OFF, (6, B), (1, 3)),
                         _ap(in1[:], AB_OFF + 3, (6, B), (1, 3)))
    nc.vector.tensor_sub(w[:, :, 1, :], _ap(in1[:], AB_OFF, (6, B), (1, 3)),
                         _ap(in1[:], AB_OFF + 3, (6, B), (1, 3)))

    # ---- S2: per-gate scalars  (tiles [B,2]) ----
    wsq = T("wsq", B, 2, 3)
    nc.vector.tensor_mul(wsq[:], w[:], w[:])
    h2 = T("h2", B, 2)
    nc.vector.tensor_reduce(out=_ap(h2[:], 0, (2, B), (1, 2), (0, 1)),
                            in_=wsq[:], axis=mybir.AxisListType.X, op=A.add)
    h = T("h", B, 2)
    nc.scalar.sqrt(h[:], h2[:])
    ih2 = T("ih2", B, 2)
    nc.vector.reciprocal(ih2[:], h2[:])
    ih = T("ih", B, 2)
    nc.vector.tensor_mul(ih[:], h[:], ih2[:])
    # range reduction: r = h - 2pi*round(h/2pi) in [-pi, pi] (magic rounding)
    MAGIC = 12582912.0
    ym = T("ym", B, 2)
    nc.vector.tensor_scalar(ym[:], h[:], 1.0 / (2 * PI), MAGIC, op0=A.mult, op1=A.add)
    rnd = T("rnd", B, 2)
    nc.vector.tensor_scalar(rnd[:], ym[:], -MAGIC, None, op0=A.add)
    rr = T("rr", B, 2)
    nc.vector.scalar_tensor_tensor(rr[:], rnd[:], -2 * PI, h[:], op0=A.mult, op1=A.add)
    sin = T("sin", B, 2)
    nc.scalar.activation(sin[:], rr[:], AF.Sin)
    ra = T("ra", B, 2)
    nc.scalar.activation(ra[:], rr[:], AF.Abs)
    # cos = sin(pi/2 - |r|) written straight into quaternion scalar slots
    pq = T("pq", B, 2, 4)
    nc.scalar.activation(_ap(pq[:], 0, (8, B), (4, 2)), ra[:], AF.Sin,
                         bias=hpi, scale=-1.0)
    snc = T("snc", B, 2)
    nc.vector.tensor_mul(snc[:], sin[:], ih[:])
    dcs = T("dcs", B, 2)
    nc.vector.tensor_sub(dcs[:], _ap(pq[:], 0, (8, B), (4, 2)), snc[:])
    s2t = T("s2t", B, 2)
    nc.vector.tensor_mul(s2t[:], dcs[:], ih2[:])

    # ---- S3: quaternion vector parts ----
    nc.vector.tensor_tensor(_ap(pq[:], 1, (8, B), (4, 2), (1, 3)),
                            _ap(snc[:], 0, (2, B), (1, 2), (0, 3)),
                            w[:], op=A.mult)

    # ---- S4: Z = (W C^T - C^T W) U  via rank-1 structure ----
    # vprod[c,j,k] = C[k, 2c] * U[k, j]; v[c,j] = sum_k
    vprod = T("vprod", B, 2, 4, 4)
    for c in range(2):
        nc.vector.tensor_tensor(vprod[:, :, c],
                                _ap(in2[:], CB_OFF + 2 * c, (16, B), (0, 4), (4, 4)),
                                _ap(in2[:], UB_OFF, (16, B), (1, 4), (4, 4)),
                                op=A.mult)
    v = T("v", B, 2, 4)
    nc.vector.tensor_reduce(out=_ap(v[:], 0, (8, B), (4, 2), (1, 4), (0, 1)),
                            in_=vprod[:], axis=mybir.AxisListType.X, op=A.add)
    # P3 (B, i4, j4, t3): t=0,1 rank-1 -pp_t*C[2t+1,:] x U[2t,:]; t=2 rows 1,3 pp*v
    P3 = T("P3", B, 4, 4, 3)
    nc.gpsimd.memset(_ap(P3[:], 2, (48, B), (3, 16)), 0.0)
    sc = T("sc", B, 2, 4)     # sc[t,i] = -pp_t * C[2t+1, i]
    nc.vector.scalar_tensor_tensor(sc[:],
                                   _ap(in1[:], PP_OFF, (2, B), (1, 2), (0, 4)), -1.0,
                                   _ap(in2[:], CB_OFF + 4, (16, B), (8, 2), (1, 4)),
                                   op0=A.mult, op1=A.mult)
    for t in range(2):
        nc.vector.tensor_tensor(_ap(P3[:], t, (48, B), (12, 4), (3, 4)),
                                _ap(sc[:], 4 * t, (8, B), (1, 4), (0, 4)),
                                _ap(in2[:], UB_OFF + 8 * t, (16, B), (0, 4), (1, 4)),
                                op=A.mult)
    # rows 1,3 of t=2 slice get pp * v
    nc.vector.tensor_tensor(_ap(P3[:], 2 + 12, (48, B), (24, 2), (3, 4)),
                            _ap(in1[:], PP_OFF, (2, B), (1, 2), (0, 4)),
                            v[:], op=A.mult)
    Z = T("Z", B, 16)
    nc.vector.tensor_reduce(out=_ap(Z[:], 0, (16, B), (1, 16), (0, 1)),
                            in_=_ap(P3[:], 0, (48, B), (3, 16), (1, 3)),
                            axis=mybir.AxisListType.X, op=A.add)

    # ---- G = Z @ R(qbar): R rows on gpsimd, terms+reduce on DVE ----
    Rq = [T(f"Rq{k}", B, 4) for k in range(4)]
    Lp = [T(f"Lp{k}", B, 4) for k in range(4)]
    for k in range(4):
        off, sA_, sB_ = XOR_AP[k]
        nc.vector.tensor_tensor(Rq[k][:],
                                _ap(pq[:], 4 + off, (8, B), (sA_, 2), (sB_, 2)),
                                _ap(cst[:], CS + 44 + 4 * k, (0, B), (1, 4)),
                                op=A.mult)
        nc.gpsimd.tensor_tensor(Lp[k][:],
                                _ap(pq[:], off, (8, B), (sA_, 2), (sB_, 2)),
                                _ap(cst[:], CS + 60 + 4 * k, (0, B), (1, 4)),
                                op=A.mult)
    Gt = T("Gt", B, 4, 16)   # (t, ij)
    Ht = T("Ht", B, 4, 16)
    for k in range(4):
        nc.vector.tensor_tensor(_ap(Gt[:], 16 * k, (64, B), (4, 4), (1, 4)),
                                _ap(Z[:], k, (16, B), (4, 4), (0, 4)),
                                _ap(Rq[k][:], 0, (4, B), (0, 4), (1, 4)),
                                op=A.mult)
        nc.gpsimd.tensor_tensor(_ap(Ht[:], 16 * k, (64, B), (4, 4), (1, 4)),
                                _ap(Lp[k][:], 0, (4, B), (1, 4), (0, 4)),
                                _ap(Z[:], 4 * k, (16, B), (0, 4), (1, 4)),
                                op=A.mult)
    Gm = T("Gm", B, 16)
    nc.vector.tensor_reduce(out=_ap(Gm[:], 0, (16, B), (1, 16), (0, 1)),
                            in_=_ap(Gt[:], 0, (64, B), (1, 16), (16, 4)),
                            axis=mybir.AxisListType.X, op=A.add, opt_input=False)
    Hs = T("Hs", B, 16)
    nc.vector.tensor_reduce(out=_ap(Hs[:], 0, (16, B), (1, 16), (0, 1)),
                            in_=_ap(Ht[:], 0, (64, B), (1, 16), (16, 4)),
                            axis=mybir.AxisListType.X, op=A.add, opt_input=False)

    # ---- kappa / lambda ----
    Gs = T("Gs", B, 16)
    nc.vector.tensor_tensor(Gs[:], Gm[:], _ap(cst[:], CS, (0, B), (1, 16)), op=A.mult)
    Hss = T("Hss", B, 16)
    nc.vector.tensor_tensor(Hss[:], Hs[:], _ap(cst[:], CS + 16, (0, B), (1, 16)),
                            op=A.mult)
    kl = T("kl", B, 2, 4)
    M1G = T("M1G", B, 8)
    M1H = T("M1H", B, 8)
    for Mt, Ss in ((M1G, Gs), (M1H, Hss)):
        nc.vector.tensor_tensor(Mt[:],
                                _ap(Ss[:], 0, (16, B), (4, 4), (2, 2)),
                                _ap(Ss[:], 5, (16, B), (8, 2), (-4, 2), (2, 2)),
                                op=A.add)
    for half, Mt in ((0, M1G), (1, M1H)):
        nc.vector.tensor_tensor(_ap(kl[:], 4 * half, (8, B), (1, 4)),
                                _ap(Mt[:], 0, (8, B), (2, 4)),
                                _ap(Mt[:], 5, (8, B), (-4, 2), (2, 2)),
                                op=A.add)

    # ---- S6: assembly ----
    pr6 = T("pr6", B, 2, 3)
    nc.vector.tensor_tensor(pr6[:], w[:], _ap(kl[:], 1, (8, B), (4, 2), (1, 3)),
                            op=A.mult)
    dot = T("dot", B, 2)
    nc.vector.tensor_reduce(out=_ap(dot[:], 0, (2, B), (1, 2), (0, 1)),
                            in_=pr6[:], axis=mybir.AxisListType.X, op=A.add)
    t6a = T("t6a", B, 2)   # snc*kl0 (note s1 = -snc)
    nc.vector.tensor_tensor(t6a[:], snc[:], _ap(kl[:], 0, (8, B), (4, 2)), op=A.mult)
    t6b = T("t6b", B, 2)
    nc.vector.tensor_mul(t6b[:], s2t[:], dot[:])
    Aq = T("Aq", B, 2)
    nc.vector.tensor_sub(Aq[:], t6b[:], t6a[:])
    tm1 = T("tm1", B, 2, 6)
    nc.vector.tensor_tensor(tm1[:], _ap(Aq[:], 0, (1, 2 * B), (0, 6)),
                            _ap(w[:], 0, (3, 2 * B), (1, 3), (0, 2)), op=A.mult)
    tm2 = T("tm2", B, 2, 6)
    nc.gpsimd.tensor_tensor(tm2[:], _ap(snc[:], 0, (1, 2 * B), (0, 6)),
                            _ap(kl[:], 1, (4, 2 * B), (1, 3), (0, 2)), op=A.mult)
    tsum = T("tsum", B, 2, 6)
    nc.vector.tensor_add(tsum[:], tm1[:], tm2[:])
    tsgn = T("tsgn", B, 2, 6)
    nc.vector.tensor_tensor(tsgn[:], tsum[:],
                            _ap(cst[:], CS + 32, (0, B), (6, 2), (1, 6)), op=A.mult)
    res = T("res", B, 6)
    nc.vector.tensor_add(res[:], tsgn[:, :, 0, :], tsgn[:, :, 1, :])
    nc.sync.dma_start(res_d, res[:].rearrange("p a b -> p (a b)"))


# ---------------- SPMD module build + host wrapper ----------------
_CACHE = {}


def _build_nc():
    nc = bacc.Bacc("TRN2", target_bir_lowering=False)
    in1_d = nc.dram_tensor("in1", [P, IN1_W], DT, kind="ExternalInput")
    res_d = nc.dram_tensor("res", [P, B * 6], DT, kind="ExternalOutput")
    from contextlib import ExitStack
    with tile.TileContext(nc) as tc:
        with ExitStack() as ctx:
            tile_body(ctx, tc, [res_d[:]], [in1_d[:]])
    if not nc.is_finalized():
        nc.finalize()
    return nc


def _prep_in_maps(chi, cov, upd, pcpa):
    g = chi.shape[0]
    k4 = cov.shape[0] // 4
    idx = np.arange(g)
    C = cov.reshape(k4, 4, k4, 4)[idx, :, idx, :].reshape(g, 16).astype(F32)
    U = upd.reshape(k4, 4, k4, 4)[idx, :, idx, :].reshape(g, 16).astype(F32)
    alpha = np.stack([chi[:, 4], -chi[:, 2], -chi[:, 3]], axis=1).astype(F32)
    beta = np.stack([chi[:, 5], -chi[:, 1], chi[:, 0]], axis=1).astype(F32)
    pe = pcpa[0::2].astype(F32)
    po = pcpa[1::2].astype(F32)
    cst = np.broadcast_to(_const_row(), (P, NC1))
    in_maps = []
    for core in range(NCORES):
        sl = slice(core * GPC, (core + 1) * GPC)
        in1 = np.empty((P, IN1_W), F32)
        abv = in1[:, AB_OFF:AB_OFF + 12].reshape(P, B, 2, 3)
        abv[:, :, 0, :] = alpha[sl].reshape(B, P, 3).transpose(1, 0, 2)
        abv[:, :, 1, :] = beta[sl].reshape(B, P, 3).transpose(1, 0, 2)
        in1[:, PP_OFF:PP_OFF + 4] = np.stack(
            [pe[sl].reshape(B, P).T, po[sl].reshape(B, P).T],
            axis=-1).reshape(P, 4)
        in1[:, CST_OFF:CST_OFF + NC1] = cst
        in1[:, CB_OFF:CB_OFF + 32] = C[sl].reshape(B, P, 16).transpose(1, 0, 2).reshape(P, 32)
        in1[:, UB_OFF:UB_OFF + 32] = U[sl].reshape(B, P, 16).transpose(1, 0, 2).reshape(P, 32)
        in_maps.append({"in1": in1})
    return in_maps


def _assemble(results, g):
    out = np.zeros((6, g), F32)
    for core in range(NCORES):
        res = results[core]["res"].reshape(P, B, 6)
        sl = slice(core * GPC, (core + 1) * GPC)
        for t in range(6):
            out[MPRIME[t], sl] = res[:, :, t].T.reshape(GPC)
    return out


def run_spmd(inputs, trace=False, **kw):
    """Run on the 8 neuron cores; returns (out (6,g) f32, BassKernelResults)."""
    if "nc" not in _CACHE:
        _CACHE["nc"] = _build_nc()
    nc = _CACHE["nc"]
    chi = np.asarray(inputs["chi"], F32)
    cov = np.asarray(inputs["covariance_matrix"], F32)
    upd = np.asarray(inputs["update_matrix"], F32)
    pcpa = np.asarray(inputs["partial_cost_partial_activation"], F32)
    in_maps = _prep_in_maps(chi, cov, upd, pcpa)
    br = run_bass_kernel_spmd(nc, in_maps, core_ids=list(range(NCORES)),
                              trace=trace, **kw)
    out = _assemble(br.results, chi.shape[0])
    return out, br


def kernel(**inputs) -> np.ndarray:
    out, _ = run_spmd(inputs, trace=False)
    return out

